# revision 2
# baseline (speedup 1.0000x reference)
"""Trainium2 Bass kernel for nn_Custom_loss_66829691125920.

Computes a CLIP-style loss: symmetric InfoNCE over max-pooled token
similarities (two image-view sets) plus a triplet margin term, on 8
NeuronCores.

Strategy
--------
- Shard the batch dim N=96 across 8 cores (12 rows each, data parallel on v).
- On the host, fold mask + 1/valid into the text tokens (max/sum commute with
  the nonneg per-token scaling), drop masked tokens, and pack the surviving
  tokens of t_pos (shared by all cores) plus each core's own t_neg tokens into
  one padded token stream of Tp = 128*Tb tokens (fp16).
- Per core, per 128-token block: PE matmuls tokens x v -> PSUM sim tiles
  [128 tok, 2*196]; VectorE max-reduces over the 196 image tokens -> word
  scores; a second PE matmul against a 0/1 segment matrix accumulates the
  per-(i, j) pooled similarities S into one PSUM tile [108, 24]
  (segments: 96 pos rows j + 12 own neg rows).
- AllGather the per-core S blocks; every core redundantly computes the final
  scalar (row/col logsumexp, diagonal, triplet relu means) on-device.
"""

import math

import numpy as np

N, P, L, D = 96, 196, 64, 128
NCORES = 8
NL = N // NCORES  # 12 rows per core
MARGIN = 0.7
CLAMP_MAX = 4.6052

_CACHE = {}


def _build_program(Tp, s, dbg=False):
    import concourse.bass as bass
    import concourse.mybir as mybir
    import concourse.tile as tile
    from concourse import bacc
    from concourse.masks import make_identity

    f32 = mybir.dt.float32
    f16 = mybir.dt.float16
    Tb = Tp // 128

    nc = bacc.Bacc("TRN2", target_bir_lowering=False, num_devices=NCORES)
    if dbg:
        d_dpay = nc.dram_tensor("dbg_pay", [128, 26], f32, kind="ExternalOutput")
        d_dsum = nc.dram_tensor("dbg_sumt", [128, 8], f32, kind="ExternalOutput")
        d_dneg = nc.dram_tensor("dbg_negd", [96, 2], f32, kind="ExternalOutput")
        d_ddiag = nc.dram_tensor("dbg_diag", [96, 2], f32, kind="ExternalOutput")
        d_dwb = nc.dram_tensor("dbg_wb", [128, 24], f32, kind="ExternalOutput")

    d_vT = nc.dram_tensor("vT", [128, 2, NL, P], f16, kind="ExternalInput")
    d_tokT = nc.dram_tensor("tokT", [128, Tp], f16, kind="ExternalInput")
    d_seg = nc.dram_tensor("seg", [128, Tb, 120], f16, kind="ExternalInput")
    d_maskN = nc.dram_tensor("maskN", [128, 12], f32, kind="ExternalInput")
    d_wvec = nc.dram_tensor("wvec", [1, 8], f32, kind="ExternalInput")
    d_out = nc.dram_tensor("loss", [1, 1], f32, kind="ExternalOutput")

    with tile.TileContext(nc) as tc:
        with (
            tc.tile_pool(name="const", bufs=1) as cpool,
            tc.tile_pool(name="word", bufs=3) as wpool,
            tc.tile_pool(name="fin", bufs=1) as fpool,
            tc.tile_pool(name="psim", bufs=2, space="PSUM") as spool,
            tc.tile_pool(name="psS", bufs=1, space="PSUM") as sppool,
            tc.tile_pool(name="dram", bufs=1, space="DRAM") as dpool,
        ):
            sb_vT = cpool.tile([128, 2, NL, P], f16)
            sb_tokT = cpool.tile([128, Tp], f16)
            sb_seg = cpool.tile([128, Tb, 120], f16)
            sb_maskN = cpool.tile([128, 12], f32)
            sb_wvec = cpool.tile([1, 8], f32)
            nc.sync.dma_start(sb_tokT[:, 0:1024], d_tokT[:, 0:1024])
            nc.scalar.dma_start(sb_vT[:, 0, :, :], d_vT[:, 0, :, :])
            nc.gpsimd.dma_start(sb_vT[:, 1, :, :], d_vT[:, 1, :, :])
            if Tp > 1024:
                mid = 1024 + (Tp - 1024) // 2
                nc.sync.dma_start(sb_tokT[:, 1024:mid], d_tokT[:, 1024:mid])
                nc.scalar.dma_start(sb_tokT[:, mid:Tp], d_tokT[:, mid:Tp])
            h1 = Tb // 2
            nc.sync.dma_start(sb_seg[:, 0:h1, :], d_seg[:, 0:h1, :])
            nc.gpsimd.dma_start(sb_seg[:, h1:Tb, :], d_seg[:, h1:Tb, :])
            nc.sync.dma_start(sb_maskN[:, :], d_maskN[:, :])
            nc.sync.dma_start(sb_wvec[:, :], d_wvec[:, :])

            # start-skew absorber: tiny collective while inputs land
            bar = fpool.tile([1, 4], f32, tag="bar")
            nc.vector.memset(bar[:, :], 0.0)
            bar_d = dpool.tile([1, 4], f32, tag="bar_d")
            gbar_d = dpool.tile([NCORES, 1, 4], f32, tag="gbar_d")
            nc.sync.dma_start(bar_d[:, :], bar[:, :])
            nc.gpsimd.collective_compute(
                "AllGather",
                mybir.AluOpType.bypass,
                replica_groups=[list(range(NCORES))],
                ins=[bar_d.opt()],
                outs=[gbar_d.opt()],
            )

            # warm the exp act-table once; PE warmup matmuls during DMA wait
            warm0 = cpool.tile([1, 1], f32, tag="warm0")
            nc.vector.memset(warm0[:, :], 1.0)
            warm1 = cpool.tile([1, 1], f32, tag="warm1")
            nc.scalar.activation(warm1[:, :], warm0[:, :], mybir.ActivationFunctionType.Exp)
            zzw = cpool.tile([128, 128], f16, tag="zzw")
            nc.vector.memset(zzw[:, :], 0.0)
            zzr = cpool.tile([128, 392], f16, tag="zzr")
            nc.vector.memset(zzr[:, :], 0.0)
            pswarm = spool.tile([128, 3, 512], f32, tag="sim")
            for w in range(26):
                nc.tensor.matmul(
                    pswarm[:, 0, 0:392], lhsT=zzw[:, :], rhs=zzr[:, :],
                    start=True, stop=True, skip_group_check=True,
                )

            # ---- main loop (vset-major): sim matmuls + max-pool + segment
            # matmul, then per-vset payload + AllGather so the first gather
            # overlaps the second vset's compute.
            # Drain: most blocks ScalarE-copies both halves into one f16
            # window; VectorE runs a consolidated TT1/TT2/reduce hierarchy.
            # Every DIRECT_K-th block (and the last) VectorE reduces both
            # halves straight from PSUM.
            DIRECT_K = 7
            g_ds = []
            for vs in range(2):
                psS = sppool.tile([120, 12], f32, tag=f"psS{vs}")
                for b in range(Tb):
                    wb = wpool.tile([128, 12], f16, tag="word")
                    vdirect = (b % DIRECT_K == DIRECT_K - 1) or (b == Tb - 1)
                    win = None
                    if not vdirect:
                        win = wpool.tile([128, 12, 196], f16, tag="win")
                    for half in range(2):
                        ps = spool.tile([128, 3, 512], f32, tag="sim")
                        for k in range(3):
                            pr = half * 3 + k
                            nc.tensor.matmul(
                                ps[:, k, 0 : 2 * P],
                                lhsT=sb_tokT[:, b * 128 : (b + 1) * 128],
                                rhs=sb_vT[:, vs, pr * 2 : pr * 2 + 2, :],
                                start=True,
                                stop=True,
                            )
                        psview = ps[:, :, 0 : 2 * P]
                        if vdirect:
                            nc.vector.tensor_reduce(
                                out=wb[:, half * 6 : half * 6 + 6],
                                in_=psview.rearrange("p a (b c) -> p a b c", c=P),
                                axis=mybir.AxisListType.X,
                                op=mybir.AluOpType.max,
                            )
                        else:
                            nc.scalar.copy(
                                win[:, 6 * half : 6 * half + 6, :].rearrange(
                                    "p r q -> p (r q)"
                                ).rearrange("p (a b) -> p a b", b=392),
                                psview,
                            )
                    if not vdirect:
                        t1 = wpool.tile([128, 12, 98], f16, tag="t1")
                        nc.vector.tensor_tensor(
                            out=t1[:, :, :],
                            in0=win[:, :, 0:98],
                            in1=win[:, :, 98:196],
                            op=mybir.AluOpType.max,
                        )
                        t2 = wpool.tile([128, 12, 49], f16, tag="t2")
                        nc.vector.tensor_tensor(
                            out=t2[:, :, :],
                            in0=t1[:, :, 0:49],
                            in1=t1[:, :, 49:98],
                            op=mybir.AluOpType.max,
                        )
                        nc.vector.tensor_reduce(
                            out=wb[:, :],
                            in_=t2[:, :, :],
                            axis=mybir.AxisListType.X,
                            op=mybir.AluOpType.max,
                        )
                    nc.tensor.matmul(
                        psS[:, :],
                        lhsT=sb_seg[:, b, :],
                        rhs=wb[:, :],
                        start=(b == 0),
                        stop=(b == Tb - 1),
                        skip_group_check=True,
                    )

                # payload: S^T block + triplet relu (rows 96-107) and diag
                # (rows 108-119) in col 12
                payload = fpool.tile([128, 13], f32, tag=f"payload{vs}")
                nc.vector.memset(payload[:, :], 0.0)
                nc.scalar.copy(payload[0:96, 0:12], psS[0:96, :])
                ntmp = fpool.tile([128, 12], f32, tag=f"ntmp{vs}")
                nc.vector.tensor_tensor(
                    out=ntmp[96:120, :],
                    in0=psS[96:120, :],
                    in1=sb_maskN[96:120, :],
                    op=mybir.AluOpType.mult,
                )
                nc.vector.tensor_reduce(
                    out=payload[96:120, 12:13],
                    in_=ntmp[96:120, :],
                    axis=mybir.AxisListType.X,
                    op=mybir.AluOpType.add,
                )
                nc.vector.tensor_scalar(
                    out=payload[96:108, 12:13],
                    in0=payload[96:108, 12:13],
                    scalar1=1.0,
                    scalar2=float(MARGIN),
                    op0=mybir.AluOpType.mult,
                    op1=mybir.AluOpType.add,
                )
                nc.vector.tensor_scalar_max(
                    payload[96:108, 12:13], payload[96:108, 12:13], 0.0
                )
                pay_d = dpool.tile([128, 13], f32, tag=f"pay{vs}")
                g_d = dpool.tile([NCORES, 128, 13], f32, tag=f"g{vs}")
                nc.sync.dma_start(pay_d[:, :], payload[:, :])
                nc.gpsimd.collective_compute(
                    "AllGather",
                    mybir.AluOpType.bypass,
                    replica_groups=[list(range(NCORES))],
                    ins=[pay_d.opt()],
                    outs=[g_d.opt()],
                )
                g_ds.append(g_d)

            # ---- final (redundant on all cores) ----
            ident = cpool.tile([128, 128], f32)
            make_identity(nc, ident[:, :])
            sb_ones = cpool.tile([128, 1], f32)
            nc.vector.memset(sb_ones[:, :], 1.0)

            sumt = fpool.tile([128, 8], f32)
            nc.vector.memset(sumt[:, :], 0.0)

            mats = []
            for vs in range(2):
                # S^T [j, i]: G[c, j, vs*12+il]
                smt = fpool.tile([96, 96], f32, tag=f"smt{vs}")
                nc.sync.dma_start(
                    smt[:, :].rearrange("j (c il) -> j c il", c=NCORES),
                    g_ds[vs][:, 0:96, 0:12].rearrange("c j il -> j c il"),
                )
                tripv = fpool.tile([1, 96], f32, tag=f"tripv{vs}")
                nc.sync.dma_start(
                    tripv[:, :].rearrange("o (c il) -> o c il", c=NCORES),
                    g_ds[vs][:, 96:108, 12:13].rearrange("c il o -> o c il"),
                )
                diagv = fpool.tile([1, 96], f32, tag=f"diagv{vs}")
                nc.sync.dma_start(
                    diagv[:, :].rearrange("o (c il) -> o c il", c=NCORES),
                    g_ds[vs][:, 108:120, 12:13].rearrange("c il o -> o c il"),
                )
                tsum = fpool.tile([1, 1], f32, tag=f"tsum{vs}")
                nc.vector.tensor_reduce(
                    out=tsum[:, :], in_=tripv[:, :],
                    axis=mybir.AxisListType.X, op=mybir.AluOpType.add,
                )
                nc.vector.tensor_scalar_mul(sumt[0:1, 6 + vs : 7 + vs], tsum[:, :], 1.0)
                dsum = fpool.tile([1, 1], f32, tag=f"dsum{vs}")
                nc.vector.tensor_reduce(
                    out=dsum[:, :], in_=diagv[:, :],
                    axis=mybir.AxisListType.X, op=mybir.AluOpType.add,
                )
                nc.vector.tensor_scalar_mul(
                    sumt[0:1, 4 + vs : 5 + vs], dsum[:, :], float(s)
                )
                # transpose -> S [i, j]
                pt = spool.tile([128, 3, 512], f32, tag="sim")
                nc.tensor.transpose(pt[0:96, 0, 0:96], smt[:, :], ident[0:96, 0:96])
                sm = fpool.tile([96, 96], f32, tag=f"sm{vs}")
                nc.scalar.copy(sm[:, :], pt[0:96, 0, 0:96])
                mats.append((1 + 2 * vs, smt))
                mats.append((0 + 2 * vs, sm))

            # logsumexp rows: max, exp(accum), newton-ln (exp-table only)
            C_LN = 1064866805.0
            K_LN = 8.262958405176314e-08
            for col, mat in mats:
                rm = fpool.tile([96, 1], f32, tag=f"rm{col}")
                nc.vector.tensor_reduce(
                    out=rm[:, :], in_=mat[:, :],
                    axis=mybir.AxisListType.X, op=mybir.AluOpType.max,
                )
                brm = fpool.tile([96, 1], f32, tag=f"brm{col}")
                nc.vector.tensor_scalar_mul(brm[:, :], rm[:, :], -float(s))
                etmp = fpool.tile([96, 96], f32, tag="etmp")
                sume = fpool.tile([96, 1], f32, tag=f"sume{col}")
                nc.scalar.activation(
                    etmp[:, :],
                    mat[:, :],
                    mybir.ActivationFunctionType.Exp,
                    bias=brm[:, :],
                    scale=float(s),
                    accum_out=sume[:, :],
                )
                iv = fpool.tile([96, 1], mybir.dt.int32, tag=f"iv{col}")
                nc.vector.tensor_scalar(
                    out=iv[:, :],
                    in0=sume[:, :].bitcast(mybir.dt.int32),
                    scalar1=int(C_LN),
                    scalar2=0,
                    op0=mybir.AluOpType.subtract,
                    op1=mybir.AluOpType.add,
                )
                lg = fpool.tile([96, 1], f32, tag=f"lg{col}")
                nc.vector.tensor_copy(lg[:, :], iv[:, :])
                nc.vector.tensor_scalar_mul(lg[:, :], lg[:, :], K_LN)
                nc.vector.scalar_tensor_tensor(
                    out=sumt[0:96, col : col + 1],
                    in0=rm[:, :],
                    scalar=float(s),
                    in1=lg[:, :],
                    op0=mybir.AluOpType.mult,
                    op1=mybir.AluOpType.add,
                )

            # column sums via ones-matmul, then weighted total
            po = spool.tile([128, 3, 512], f32, tag="sim")
            nc.tensor.matmul(
                po[0:1, 0, 0:8], lhsT=sb_ones[:, :], rhs=sumt[:, :], start=True, stop=True
            )
            so = fpool.tile([1, 8], f32, tag="so")
            nc.scalar.copy(so[:, :], po[0:1, 0, 0:8])
            sw = fpool.tile([1, 8], f32, tag="sw")
            nc.vector.tensor_tensor(
                out=sw[:, :], in0=so[:, :], in1=sb_wvec[:, :], op=mybir.AluOpType.mult
            )
            res = fpool.tile([1, 1], f32, tag="res")
            nc.vector.tensor_reduce(
                out=res[:, :],
                in_=sw[:, :],
                axis=mybir.AxisListType.X,
                op=mybir.AluOpType.add,
            )
            nc.sync.dma_start(d_out[:, :], res[:, :])

    nc.compile()
    return nc


def _prepare_inputs(inputs):
    v_main = np.asarray(inputs["v_main"], np.float32)
    v_aug = np.asarray(inputs["v_aug"], np.float32)
    t_pos = np.asarray(inputs["t_pos"], np.float32)
    t_neg = np.asarray(inputs["t_neg"], np.float32)
    m_pos = np.asarray(inputs["m_pos"]).astype(bool)
    m_neg = np.asarray(inputs["m_neg"]).astype(bool)
    ls = float(np.asarray(inputs["logit_scale"], np.float32))
    s = float(np.exp(np.clip(ls, 0.0, CLAMP_MAX)))

    valid_pos = np.maximum(m_pos.sum(1), 1).astype(np.float32)
    valid_neg = np.maximum(m_neg.sum(1), 1).astype(np.float32)
    jj, llp = np.nonzero(m_pos)
    Kpos = len(jj)
    pos_tok = t_pos[jj, llp, :] / valid_pos[jj][:, None]
    nii, nll = np.nonzero(m_neg)
    maxKn = max(int(np.sum((nii // NL) == c)) for c in range(NCORES)) if len(nii) else 0
    Tp = 128 * int(math.ceil((Kpos + maxKn) / 128)) if (Kpos + maxKn) else 128
    Tb = Tp // 128

    maskN = np.zeros((128, 12), np.float32)
    for il in range(12):
        maskN[96 + il, il] = 1.0
        maskN[108 + il, il] = 1.0
    wvec = (np.array([[1, 1, 1, 1, -2, -2, 1, 1]], np.float32) / 192.0)

    in_maps = []
    for c in range(NCORES):
        tok = np.zeros((Tp, D), np.float32)
        seg = np.zeros((Tp, 120), np.float32)
        tok[:Kpos] = pos_tok
        seg[np.arange(Kpos), jj] = 1.0
        own = (jj // NL) == c
        oidx = np.nonzero(own)[0]
        seg[oidx, 96 + jj[oidx] % NL] = -1.0
        seg[oidx, 108 + jj[oidx] % NL] = 1.0
        sel = (nii // NL) == c
        ii, lln = nii[sel], nll[sel]
        kneg = len(ii)
        tok[Kpos : Kpos + kneg] = t_neg[ii, lln, :] / valid_neg[ii][:, None]
        seg[Kpos + np.arange(kneg), 96 + ii % NL] = 1.0

        rows = slice(c * NL, (c + 1) * NL)
        vv = np.stack([v_main[rows], v_aug[rows]])  # [2,12,196,128]
        in_maps.append(
            {
                "vT": np.ascontiguousarray(np.transpose(vv, (3, 0, 1, 2))).astype(
                    np.float16
                ),
                "tokT": np.ascontiguousarray(tok.T).astype(np.float16),
                "seg": np.ascontiguousarray(
                    np.transpose(seg.reshape(Tb, 128, 120), (1, 0, 2))
                ).astype(np.float16),
                "maskN": maskN,
                "wvec": wvec,
            }
        )
    return in_maps, Tp, s


def kernel(_trace=False, **inputs):
    from concourse.bass_utils import run_bass_kernel_spmd

    in_maps, Tp, s = _prepare_inputs(inputs)

    key = (Tp, round(s, 9))
    nc = _CACHE.get(key)
    if nc is None:
        nc = _build_program(Tp, s)
        _CACHE[key] = nc

    br = None
    for attempt in range(3):
        try:
            br = run_bass_kernel_spmd(
                nc, in_maps, core_ids=list(range(NCORES)), trace=_trace
            )
            break
        except ModuleNotFoundError:
            # no axon NTFF hook in this container -> run untraced
            _trace = False
        except Exception:
            # transient NRT_EXEC_UNIT_UNRECOVERABLE on the axon terminal has
            # been observed between back-to-back loads; a retry recovers it
            if attempt == 2:
                raise
            import time as _time

            _time.sleep(5.0)
    assert br is not None
    if _trace and br.exec_time_ns is not None:
        kernel.last_exec_time_ns = br.exec_time_ns
    loss = br.results[0]["loss"]
    return np.asarray(loss, np.float32).reshape(())


kernel.last_exec_time_ns = None

